# revision 1
# baseline (speedup 1.0000x reference)
"""Trainium2 Bass kernel for nn_NodeModel (GNN message passing).

  out = relu(concat([x, scatter_mean(edge_attr, col), u[batch]]) @ W1 + b1) @ W2 + b2

Strategy (8 NeuronCores, data-parallel over destination nodes):
  * Host: sort edges by destination node (col). Every node has degree <= 64
    (data max is 58), so each node's edges are padded to exactly DEG=64
    "edge slots"; edge values are pre-scaled by 1/count so the sum over
    slots directly yields scatter_mean. Nodes are partitioned contiguously
    across the 8 cores (12500 nodes/core -> 100 windows of 128 node slots).
  * Device, per core: a GPSIMD accumulate-DMA streams the DEG edge-slot
    planes from HBM and sums them into an SBUF tile gsn[128 nodes, 16]
    per window (the segment reduction happens inside the DMA engines).
    A PE transpose turns gsn into e_aggT[16, 128], then the MLP runs with
    nodes on the free dim: psH = W1e.T@e_aggT + W1xu.T@xuT (PSUM),
    relu+bias on ACT, psO = W2.T@hid, bias on ACT, DMA out.
  * No cross-core communication: edges live with their destination node.
"""

import numpy as np

try:
    import ml_dtypes

    _BF16 = np.dtype(ml_dtypes.bfloat16)
except Exception:  # pragma: no cover
    _BF16 = None

F_E, F_X, F_U, H, F_OUT = 16, 64, 64, 128, 64
XU = F_X + F_U  # 128

CFG = dict(
    n_cores=8,
    npc=12500,   # real nodes per core
    wpc=100,     # windows (128 node slots) per core
    chw=20,      # windows per edge-stream chunk
    b=4,         # windows per MLP batch group
    deg=64,      # padded degree (edge slots per node)
    use_accum_dma=False,
    pool_split=False,  # pre-add edge-slot halves on GpSimd before DVE reduce
    edge_dt="bf16",
    xu_dt="bf16",
    w_dt="bf16",
)

_CACHE = {}


def _npdt(name):
    return _BF16 if name == "bf16" else np.dtype(np.float32)


def _mydt(name, mybir):
    return mybir.dt.bfloat16 if name == "bf16" else mybir.dt.float32


# ---------------------------------------------------------------- host side
def _preprocess(inputs, cfg):
    NC, NPC, WPC, CHW, DEG = (
        cfg["n_cores"], cfg["npc"], cfg["wpc"], cfg["chw"], cfg["deg"],
    )
    NCH = WPC // CHW
    SLOTS = WPC * 128
    edt = _npdt(cfg["edge_dt"])
    xdt = _npdt(cfg["xu_dt"])
    wdt = _npdt(cfg["w_dt"])

    x = np.asarray(inputs["x"], np.float32)
    ea = np.asarray(inputs["edge_attr"], np.float32)
    u = np.asarray(inputs["u"], np.float32)
    W1 = np.asarray(inputs["W1"], np.float32)
    b1 = np.asarray(inputs["b1"], np.float32)
    W2 = np.asarray(inputs["W2"], np.float32)
    b2 = np.asarray(inputs["b2"], np.float32)
    col = np.asarray(np.asarray(inputs["edge_index"])[1], np.int64)
    batch = np.asarray(inputs["batch"], np.int64)

    N, E = x.shape[0], col.shape[0]
    assert N == NC * NPC, (N, NC, NPC)

    cnt = np.bincount(col, minlength=N)
    assert cnt.max() <= DEG, f"max degree {cnt.max()} > DEG {DEG}"
    invc = np.zeros(N, np.float32)
    nz = cnt > 0
    invc[nz] = 1.0 / cnt[nz]

    order = np.argsort(col, kind="stable")
    cols = col[order]
    eas = ea[order] * invc[cols][:, None]  # pre-scaled by 1/count

    starts = np.concatenate([[0], np.cumsum(cnt)[:-1]])
    rank = np.arange(E, dtype=np.int64) - starts[cols]  # slot within node
    c = cols // NPC
    m = cols - c * NPC
    w = m >> 7          # window within core
    p = m & 127         # node slot within window
    ch = w // CHW
    wi = w - ch * CHW

    if cfg["use_accum_dma"]:
        # layout [core][chunk][slot e][p][wi*16+f]
        A = np.zeros((NC, NCH, DEG, 128, CHW * F_E), edt)
        rows = (((c * NCH + ch) * DEG + rank) * 128 + p) * CHW + wi
        A.reshape(-1, F_E)[rows] = eas.astype(edt)
    else:
        # layout [core][w][p][f][e]
        tmp = np.zeros((NC, WPC, 128, DEG, F_E), edt)
        rows = ((c * WPC + w) * 128 + p) * DEG + rank
        tmp.reshape(-1, F_E)[rows] = eas.astype(edt)
        A = np.ascontiguousarray(tmp.swapaxes(3, 4))  # [NC, WPC, 128, 16, DEG]

    # node features: concat(x, u[batch]) transposed, padded to SLOTS
    xu = np.concatenate([x, u[batch]], axis=1)  # [N, 128]
    xuT = np.zeros((NC, XU, SLOTS), xdt)
    xr = xu.reshape(NC, NPC, XU)
    for ci in range(NC):
        xuT[ci, :, :NPC] = xr[ci].T.astype(xdt)

    W1xu = np.ascontiguousarray(
        np.concatenate([W1[0:F_X], W1[F_X + F_E:]], axis=0), dtype=wdt
    )  # [128, 128]
    W1e = np.ascontiguousarray(W1[F_X:F_X + F_E], dtype=wdt)  # [16, 128]
    W2c = np.ascontiguousarray(W2, dtype=wdt)  # [128, 64]
    ident = np.eye(128, dtype=np.float32)

    common = dict(
        w1xu=W1xu, w1e=W1e, w2=W2c,
        b1=np.ascontiguousarray(b1.reshape(H, 1), np.float32),
        b2=np.ascontiguousarray(b2.reshape(F_OUT, 1), np.float32),
        ident=ident,
    )
    in_maps = []
    for ci in range(NC):
        im = dict(common)
        im["edges"] = A[ci]
        im["xut"] = xuT[ci]
        in_maps.append(im)
    return in_maps


def _postprocess(results, cfg):
    NC, NPC, WPC, B = cfg["n_cores"], cfg["npc"], cfg["wpc"], cfg["b"]
    SLOTS = WPC * 128
    out = np.empty((NC * NPC, F_OUT), np.float32)
    for ci in range(NC):
        o = np.asarray(results[ci]["outT"])  # [NB, 64, B*128]
        o = o.transpose(1, 0, 2).reshape(F_OUT, SLOTS)
        out[ci * NPC:(ci + 1) * NPC] = o[:, :NPC].T
    return out


# ------------------------------------------------------------- device side
def _build(cfg):
    import concourse.bacc as bacc
    import concourse.bass as bass
    import concourse.mybir as mybir
    import concourse.tile as tile
    from contextlib import ExitStack

    NC, WPC, CHW, B, DEG = (
        cfg["n_cores"], cfg["wpc"], cfg["chw"], cfg["b"], cfg["deg"],
    )
    NCH = WPC // CHW
    NB = WPC // B
    GPB = CHW // B  # B-groups per chunk
    SLOTS = WPC * 128
    f32 = mybir.dt.float32
    edt = _mydt(cfg["edge_dt"], mybir)
    xdt = _mydt(cfg["xu_dt"], mybir)
    wdt = _mydt(cfg["w_dt"], mybir)
    AF = mybir.ActivationFunctionType

    nc = bacc.Bacc("TRN2", target_bir_lowering=False)

    if cfg["use_accum_dma"]:
        edges_d = nc.dram_tensor(
            "edges", [NCH, DEG, 128, CHW * F_E], edt, kind="ExternalInput")
    else:
        edges_d = nc.dram_tensor(
            "edges", [WPC, 128, F_E, DEG], edt, kind="ExternalInput")
    xut_d = nc.dram_tensor("xut", [XU, SLOTS], xdt, kind="ExternalInput")
    w1xu_d = nc.dram_tensor("w1xu", [XU, H], wdt, kind="ExternalInput")
    w1e_d = nc.dram_tensor("w1e", [F_E, H], wdt, kind="ExternalInput")
    w2_d = nc.dram_tensor("w2", [H, F_OUT], wdt, kind="ExternalInput")
    b1_d = nc.dram_tensor("b1", [H, 1], f32, kind="ExternalInput")
    b2_d = nc.dram_tensor("b2", [F_OUT, 1], f32, kind="ExternalInput")
    ident_d = nc.dram_tensor("ident", [128, 128], f32, kind="ExternalInput")
    out_d = nc.dram_tensor("outT", [NB, F_OUT, B * 128], f32,
                           kind="ExternalOutput")

    with tile.TileContext(nc) as tc, ExitStack() as ctx:
        consts = ctx.enter_context(tc.tile_pool(name="consts", bufs=1))
        gsn_pool = ctx.enter_context(tc.tile_pool(name="gsn", bufs=2))
        ea_pool = ctx.enter_context(tc.tile_pool(name="ea", bufs=2))
        hid_pool = ctx.enter_context(tc.tile_pool(name="hid", bufs=2))
        out_pool = ctx.enter_context(tc.tile_pool(name="outs", bufs=3))
        pse_pool = ctx.enter_context(
            tc.tile_pool(name="pse", bufs=2, space="PSUM"))
        psh_pool = ctx.enter_context(
            tc.tile_pool(name="psh", bufs=2, space="PSUM"))
        pso_pool = ctx.enter_context(
            tc.tile_pool(name="pso", bufs=2, space="PSUM"))
        if not cfg["use_accum_dma"]:
            edge_pool = ctx.enter_context(tc.tile_pool(name="edges", bufs=4))
            if cfg.get("pool_split"):
                tmp_pool = ctx.enter_context(tc.tile_pool(name="tmph", bufs=3))

        ident_t = consts.tile([128, 128], f32)
        nc.sync.dma_start(ident_t[:], ident_d[:])
        w1xu_t = consts.tile([XU, H], wdt)
        nc.sync.dma_start(w1xu_t[:], w1xu_d[:])
        w1e_t = consts.tile([F_E, H], wdt)
        nc.sync.dma_start(w1e_t[:], w1e_d[:])
        w2_t = consts.tile([H, F_OUT], wdt)
        nc.sync.dma_start(w2_t[:], w2_d[:])
        b1_t = consts.tile([H, 1], f32)
        nc.sync.dma_start(b1_t[:], b1_d[:])
        b2_t = consts.tile([F_OUT, 1], f32)
        nc.sync.dma_start(b2_t[:], b2_d[:])
        xut_t = consts.tile([XU, SLOTS], xdt)
        nc.sync.dma_start(xut_t[:], xut_d[:])

        for chi in range(NCH):
            gsn = gsn_pool.tile([128, CHW * F_E], f32)
            if cfg["use_accum_dma"]:
                nc.gpsimd.memset(gsn[:], 0.0)
                src = edges_d[chi].rearrange("e p f -> p e f")
                dst_ap = gsn[:]
                dst = bass.AP(
                    dst_ap.tensor, dst_ap.offset,
                    [dst_ap.ap[0], [0, DEG]] + dst_ap.ap[1:],
                )
                nc.gpsimd.dma_start(dst, src, accum_op=mybir.AluOpType.add)
            else:
                for wi in range(CHW):
                    wg = chi * CHW + wi
                    et = edge_pool.tile([128, F_E * DEG], edt)
                    nc.sync.dma_start(
                        et[:], edges_d[wg].rearrange("p f e -> p (f e)"))
                    ev = et[:].rearrange("p (f e) -> p f e", e=DEG)
                    if cfg.get("pool_split"):
                        hd = DEG // 2
                        tmp = tmp_pool.tile([128, F_E * hd], edt)
                        tv = tmp[:].rearrange("p (f e) -> p f e", e=hd)
                        nc.gpsimd.scalar_tensor_tensor(
                            out=tv, in0=ev[:, :, 0:hd], scalar=1.0,
                            in1=ev[:, :, hd:DEG],
                            op0=mybir.AluOpType.mult,
                            op1=mybir.AluOpType.add,
                        )
                        ev = tv
                    nc.vector.tensor_reduce(
                        out=gsn[:, wi * F_E:(wi + 1) * F_E],
                        in_=ev,
                        axis=mybir.AxisListType.X,
                        op=mybir.AluOpType.add,
                    )

            for bi in range(GPB):
                g = chi * GPB + bi
                pse = pse_pool.tile([F_E, B * 128], f32)
                for j in range(B):
                    wi = bi * B + j
                    nc.tensor.transpose(
                        pse[:, j * 128:(j + 1) * 128],
                        gsn[:, wi * F_E:(wi + 1) * F_E],
                        ident_t[:],
                    )
                ea = ea_pool.tile([F_E, B * 128], wdt)
                nc.vector.tensor_copy(ea[:], pse[:])

                psh = psh_pool.tile([H, B * 128], f32)
                for j in range(B):
                    wg = g * B + j
                    nc.tensor.matmul(
                        psh[:, j * 128:(j + 1) * 128],
                        w1e_t[:], ea[:, j * 128:(j + 1) * 128],
                        start=True, stop=False,
                    )
                    nc.tensor.matmul(
                        psh[:, j * 128:(j + 1) * 128],
                        w1xu_t[:], xut_t[:, wg * 128:(wg + 1) * 128],
                        start=False, stop=True,
                    )
                hid = hid_pool.tile([H, B * 128], wdt)
                nc.scalar.activation(hid[:], psh[:], AF.Relu,
                                     bias=b1_t[:], scale=1.0)

                pso = pso_pool.tile([F_OUT, B * 128], f32)
                for j in range(B):
                    nc.tensor.matmul(
                        pso[:, j * 128:(j + 1) * 128],
                        w2_t[:], hid[:, j * 128:(j + 1) * 128],
                        start=True, stop=True,
                    )
                outs = out_pool.tile([F_OUT, B * 128], f32)
                nc.scalar.activation(outs[:], pso[:], AF.Identity,
                                     bias=b2_t[:], scale=1.0)
                nc.sync.dma_start(out_d[g], outs[:])

    nc.finalize()
    return nc


def _get_program(cfg):
    key = tuple(sorted((k, v) for k, v in cfg.items()))
    if key not in _CACHE:
        _CACHE[key] = _build(cfg)
    return _CACHE[key]


def run(inputs, cfg=None, trace=False):
    from concourse.bass_utils import run_bass_kernel_spmd

    cfg = dict(CFG if cfg is None else cfg)
    nc = _get_program(cfg)
    in_maps = _preprocess(inputs, cfg)
    res = run_bass_kernel_spmd(
        nc, in_maps, list(range(cfg["n_cores"])), trace=trace)
    out = _postprocess(res.results, cfg)
    return out, res


def kernel(**inputs):
    return run(inputs)[0]



# revision 5
# speedup vs baseline: 98862.0571x; 98862.0571x over previous
"""Trainium2 Bass kernel v2 for nn_NodeModel (GNN message passing).

  out = relu(concat([x, scatter_mean(edge_attr, col), u[batch]]) @ W1 + b1) @ W2 + b2

Strategy (8 NeuronCores, data-parallel over destination nodes):
  * Host: sort nodes GLOBALLY by in-degree, deal ranks round-robin to the
    8 cores (rank r -> core r%8, slot r//8) so every core sees the same
    degree profile and one compiled schedule serves all cores (SPMD).
  * Nodes are grouped in windows of 128 slots; 8 windows = one DMA group
    (16 feats x 8 windows = 128 partitions). Each group's edge slots are
    padded only to that group's max degree D_g (data max ~58, mean 32),
    nearly halving HBM traffic vs a global pad-to-64.
  * Edge values are pre-scaled by 1/deg on host so a plain sum gives the
    scatter_mean. Device: one DMA + one DVE tensor_reduce per group
    produces e_aggT[16f x 128n] blocks directly on the right partitions
    (bf16 out -> DVE 2x mode, no PE transposes, no PSUM round-trip).
  * MLP per 4 windows (512 node slots): psH = W1xu.T@xuT (N=512) +
    W1e.T@e_aggT per window; ReLU+b1 on ACT; psO = W2.T@hid; +b2; DMA out.
  * No cross-core communication: edges live with their destination node.
"""

import numpy as np

try:
    import ml_dtypes

    _BF16 = np.dtype(ml_dtypes.bfloat16)
except Exception:  # pragma: no cover
    _BF16 = None

F_E, F_X, F_U, H, F_OUT = 16, 64, 64, 128, 64
XU = F_X + F_U        # 128
N_NODES = 100000
N_GRAPHS = 64
NC_ = 8               # cores
NPC = 12500           # real nodes per core
WPC = 100             # windows of 128 slots per core
SLOTS = WPC * 128     # 12800
GW = 8                # windows per DMA group
NG = (WPC + GW - 1) // GW  # 13 groups (12 full + 1 of 4 windows)
BW = 4                # windows per MLP batch
NB = WPC // BW        # 25

CFG = dict(
    repeat=1,        # timing only: run body N times inside the NEFF
    gs_dt="bf16",    # e_agg reduce output dtype ("bf16" enables DVE 2x)
    out_dt="bf16",   # device output dtype
    pad4=2,          # round window max degree up to this multiple
)

_CACHE = {}


def _npdt(name):
    return _BF16 if name == "bf16" else np.dtype(np.float32)


def _to_bf16_bits(x):
    """f32 -> bf16 bit pattern (uint16), round-to-nearest-even. Much faster
    than numpy casting through ml_dtypes."""
    b = np.ascontiguousarray(x, np.float32).view(np.uint32)
    rnd = ((b >> 16) & 1) + np.uint32(0x7FFF)
    return ((b + rnd) >> 16).astype(np.uint16)


def _schedule(cnt, pad4):
    """Shared degree schedule.

    Nodes get global ranks by ascending degree; windows of 1024 ranks are
    then re-ordered so each group of 8 windows has (near-)equal padded max
    degree. Returns the node->rank map and per-group degrees/offsets.
    """
    order = np.argsort(cnt, kind="stable")  # node ids by ascending degree
    deg_sorted = np.zeros(NC_ * SLOTS, cnt.dtype)
    deg_sorted[:N_NODES] = cnt[order]
    # window w covers global ranks [1024w, 1024(w+1)); max degree there
    wmax = deg_sorted.reshape(WPC, 8 * 128).max(axis=1)
    wmax = np.maximum(wmax, 1)
    wmax = ((wmax + pad4 - 1) // pad4) * pad4
    wo = np.argsort(wmax, kind="stable")    # window positions by padded deg
    wpos = np.empty(WPC, np.int64)
    wpos[wo] = np.arange(WPC, dtype=np.int64)
    dg = np.zeros(NG, np.int64)
    for g in range(NG):
        dg[g] = wmax[wo[g * GW:min((g + 1) * GW, WPC)]].max()
    offs = np.zeros(NG + 1, np.int64)
    for g in range(NG):
        offs[g + 1] = offs[g] + 128 * dg[g]

    # node -> new global rank (window-reordered degree rank)
    invperm = np.empty(N_NODES, np.int64)
    invperm[order] = np.arange(N_NODES, dtype=np.int64)
    r = invperm
    c = r & 7
    s = r >> 3
    w = s >> 7
    n = s & 127
    rk2 = ((wpos[w] << 7 | n) << 3) | c
    return rk2, dg, offs


# ---------------------------------------------------------------- host side
def _preprocess(inputs, cfg):
    edt = _npdt("bf16")

    x = np.asarray(inputs["x"], np.float32)
    ea = np.asarray(inputs["edge_attr"], np.float32)
    u = np.asarray(inputs["u"], np.float32)
    W1 = np.asarray(inputs["W1"], np.float32)
    b1 = np.asarray(inputs["b1"], np.float32)
    W2 = np.asarray(inputs["W2"], np.float32)
    b2 = np.asarray(inputs["b2"], np.float32)
    col = np.asarray(np.asarray(inputs["edge_index"])[1], np.int64)
    batch = np.asarray(inputs["batch"], np.int64)

    N, E = x.shape[0], col.shape[0]
    assert N == N_NODES

    cnt = np.bincount(col, minlength=N)
    rk2, dg, offs = _schedule(cnt, cfg["pad4"])
    ftot = int(offs[-1])

    invc = np.zeros(N, np.float32)
    nz = cnt > 0
    invc[nz] = 1.0 / cnt[nz]

    order = np.argsort(col, kind="stable")
    cols = col[order]
    eas = _to_bf16_bits(ea[order] * invc[cols][:, None])  # pre-scaled

    starts = np.concatenate([[0], np.cumsum(cnt)[:-1]])
    rank = np.arange(E, dtype=np.int64) - starts[cols]   # slot within node

    r = rk2[cols]
    c = r & 7
    s = r >> 3
    w = s >> 7
    n = s & 127
    g = np.minimum(w >> 3, NG - 1)
    wl = w - (g << 3)
    base = ((c * 128 + wl * 16) * ftot + offs[g] + n * dg[g] + rank).astype(
        np.int64)
    EBUF = np.zeros((NC_ * 128, ftot), np.uint16)
    flat = EBUF.reshape(-1)
    for f in range(F_E):
        flat[base + f * ftot] = eas[:, f]
    EBUF = EBUF.view(edt)

    # node features: concat(x, u[batch]) scattered to rank slots, transposed
    xu_bits = _to_bf16_bits(np.concatenate([x, u[batch]], axis=1))  # [N, 128]
    xu2 = np.zeros((NC_, SLOTS, XU), np.uint16)
    xu2[rk2 & 7, rk2 >> 3] = xu_bits
    xuT = np.ascontiguousarray(xu2.transpose(0, 2, 1)).view(edt)

    W1xu = np.ascontiguousarray(
        np.concatenate([W1[0:F_X], W1[F_X + F_E:]], axis=0), dtype=edt
    )  # [128, 128]
    W1e = W1[F_X:F_X + F_E]  # [16, 128]
    # zero-padded replicated copies: variant q holds w1e at rows
    # [q*16, q*16+16) of each 64-row block, so each window's K=64 edge
    # matmul reads lhsT/rhs at the same {0,64}-aligned base partition
    W1eq = np.zeros((4, 128, H), np.float32)
    for q in range(4):
        for b in range(2):
            W1eq[q, 64 * b + 16 * q:64 * b + 16 * q + 16] = W1e
    W2c = np.ascontiguousarray(W2, dtype=edt)  # [128, 64]

    common = dict(
        w1xu=W1xu, w1eq=W1eq.astype(edt), w2=W2c,
        b1=np.ascontiguousarray(b1.reshape(H, 1), np.float32),
        b2=np.ascontiguousarray(b2.reshape(F_OUT, 1), np.float32),
    )
    in_maps = []
    for ci in range(NC_):
        im = dict(common)
        im["edges"] = EBUF[ci * 128:(ci + 1) * 128]
        im["xut"] = xuT[ci]
        in_maps.append(im)
    meta = dict(rk2=rk2, dg=tuple(int(v) for v in dg),
                offs=tuple(int(v) for v in offs))
    return in_maps, meta


def _postprocess(results, meta, cfg):
    arr = np.stack(
        [np.asarray(results[ci]["outT"]).astype(np.float32)
         for ci in range(NC_)])
    # [8, NB, 64, 512] -> per core [64, SLOTS] -> rank-major rows
    o = arr.transpose(0, 2, 1, 3).reshape(NC_, F_OUT, SLOTS)
    R = o.transpose(2, 0, 1).reshape(SLOTS * NC_, F_OUT)  # row r = s*8+c
    return np.ascontiguousarray(R[meta["rk2"]], dtype=np.float32)


# ------------------------------------------------------------- device side
def _build(cfg, dg, offs):
    import concourse.bacc as bacc
    import concourse.mybir as mybir
    import concourse.tile as tile
    from contextlib import ExitStack

    f32 = mybir.dt.float32
    bf16 = mybir.dt.bfloat16
    gdt = bf16 if cfg["gs_dt"] == "bf16" else f32
    odt = f32 if cfg["out_dt"] == "f32" else bf16
    AF = mybir.ActivationFunctionType
    ftot = int(offs[-1])

    nc = bacc.Bacc("TRN2", target_bir_lowering=False)

    edges_d = nc.dram_tensor("edges", [128, ftot], bf16, kind="ExternalInput")
    xut_d = nc.dram_tensor("xut", [XU, SLOTS], bf16, kind="ExternalInput")
    w1xu_d = nc.dram_tensor("w1xu", [XU, H], bf16, kind="ExternalInput")
    w1eq_d = nc.dram_tensor("w1eq", [4, 128, H], bf16, kind="ExternalInput")
    w2_d = nc.dram_tensor("w2", [H, F_OUT], bf16, kind="ExternalInput")
    b1_d = nc.dram_tensor("b1", [H, 1], f32, kind="ExternalInput")
    b2_d = nc.dram_tensor("b2", [F_OUT, 1], f32, kind="ExternalInput")
    out_d = nc.dram_tensor("outT", [NB, F_OUT, BW * 128], odt,
                           kind="ExternalOutput")

    with tile.TileContext(nc) as tc, ExitStack() as ctx:
        consts = ctx.enter_context(tc.tile_pool(name="consts", bufs=1))
        edge_pool = ctx.enter_context(tc.tile_pool(name="edges", bufs=3))
        gs_pool = ctx.enter_context(tc.tile_pool(name="gs", bufs=2))
        hid_pool = ctx.enter_context(tc.tile_pool(name="hid", bufs=2))
        out_pool = ctx.enter_context(tc.tile_pool(name="outs", bufs=3))
        psh_pool = ctx.enter_context(
            tc.tile_pool(name="psh", bufs=2, space="PSUM"))
        pso_pool = ctx.enter_context(
            tc.tile_pool(name="pso", bufs=2, space="PSUM"))

        w1xu_t = consts.tile([XU, H], bf16)
        nc.sync.dma_start(w1xu_t[:], w1xu_d[:])
        w1eq_t = []
        for q in range(4):
            wq = consts.tile([128, H], bf16, name=f"w1eq{q}")
            nc.sync.dma_start(wq[:], w1eq_d[q])
            w1eq_t.append(wq)
        w2_t = consts.tile([H, F_OUT], bf16)
        nc.sync.dma_start(w2_t[:], w2_d[:])
        b1_t = consts.tile([H, 1], f32)
        nc.sync.dma_start(b1_t[:], b1_d[:])
        b2_t = consts.tile([F_OUT, 1], f32)
        nc.sync.dma_start(b2_t[:], b2_d[:])
        xut_t = consts.tile([XU, SLOTS], bf16)
        nc.sync.dma_start(xut_t[:], xut_d[:])

        def _group_body(g):
            D = int(dg[g])
            off = int(offs[g])
            gwg = min(GW, WPC - g * GW)       # 8, last group 4
            P = gwg * 16
            et = edge_pool.tile([128, 128 * D], bf16)
            nc.sync.dma_start(et[0:P, :], edges_d[0:P, off:off + 128 * D])
            gsb = gs_pool.tile([128, 128], gdt)
            ev = et[0:P, :].rearrange("p (n d) -> p n d", d=D)
            if gdt == bf16:
                with nc.allow_low_precision(
                        reason="DVE reduce accumulates fp32 internally"):
                    nc.vector.tensor_reduce(
                        out=gsb[0:P, :], in_=ev,
                        axis=mybir.AxisListType.X, op=mybir.AluOpType.add)
            else:
                nc.vector.tensor_reduce(
                    out=gsb[0:P, :], in_=ev,
                    axis=mybir.AxisListType.X, op=mybir.AluOpType.add)

            for half in range(gwg // BW):
                mg = g * 2 + half
                psh = psh_pool.tile([H, BW * 128], f32)
                nc.tensor.matmul(
                    psh[:, :], w1xu_t[:],
                    xut_t[:, mg * 512:(mg + 1) * 512],
                    start=True, stop=False)
                for j in range(BW):
                    wl = half * BW + j
                    blk, q = divmod(wl, 4)
                    nc.tensor.matmul(
                        psh[:, j * 128:(j + 1) * 128],
                        w1eq_t[q][blk * 64:(blk + 1) * 64, :],
                        gsb[blk * 64:(blk + 1) * 64, :],
                        start=False, stop=True)
                hid = hid_pool.tile([H, BW * 128], bf16)
                nc.scalar.activation(hid[:], psh[:], AF.Relu,
                                     bias=b1_t[:], scale=1.0)
                pso = pso_pool.tile([F_OUT, BW * 128], f32)
                nc.tensor.matmul(pso[:], w2_t[:], hid[:],
                                 start=True, stop=True)
                outs = out_pool.tile([F_OUT, BW * 128], odt)
                nc.scalar.activation(outs[:], pso[:], AF.Identity,
                                     bias=b2_t[:], scale=1.0)
                nc.sync.dma_start(out_d[mg], outs[:])

        rep = cfg.get("repeat", 1)
        if rep > 1:
            with tc.For_i(0, rep, 1):
                for g in range(NG):
                    _group_body(g)
        else:
            for g in range(NG):
                _group_body(g)

    nc.finalize()
    return nc


def _get_program(cfg, dg, offs):
    key = (tuple(sorted(cfg.items())), tuple(dg), tuple(offs))
    if key not in _CACHE:
        _CACHE[key] = _build(cfg, dg, offs)
    return _CACHE[key]


def run(inputs, cfg=None, trace=False):
    from concourse.bass_utils import run_bass_kernel_spmd

    cfg = dict(CFG if cfg is None else cfg)
    in_maps, meta = _preprocess(inputs, cfg)
    nc = _get_program(cfg, meta["dg"], meta["offs"])
    res = run_bass_kernel_spmd(nc, in_maps, list(range(NC_)), trace=trace)
    out = _postprocess(res.results, meta, cfg)
    return out, res, (nc, in_maps, meta)


def kernel(**inputs):
    return run(inputs)[0]


# revision 35
# speedup vs baseline: 108309.9927x; 1.0956x over previous
"""Trainium2 Bass kernel v2 for nn_NodeModel (GNN message passing).

  out = relu(concat([x, scatter_mean(edge_attr, col), u[batch]]) @ W1 + b1) @ W2 + b2

Strategy (8 NeuronCores, data-parallel over destination nodes):
  * Host: sort nodes GLOBALLY by in-degree, deal ranks round-robin to the
    8 cores (rank r -> core r%8, slot r//8) so every core sees the same
    degree profile and one compiled schedule serves all cores (SPMD).
  * Nodes are grouped in windows of 128 slots; 8 windows = one DMA group
    (16 feats x 8 windows = 128 partitions). Each group's edge slots are
    padded only to that group's max degree D_g (data max ~58, mean 32),
    nearly halving HBM traffic vs a global pad-to-64.
  * Edge values are pre-scaled by 1/deg on host so a plain sum gives the
    scatter_mean. Device: one DMA + one DVE tensor_reduce per group
    produces e_aggT[16f x 128n] blocks directly on the right partitions
    (bf16 out -> DVE 2x mode, no PE transposes, no PSUM round-trip).
  * MLP per 4 windows (512 node slots): psH = W1xu.T@xuT (N=512) +
    W1e.T@e_aggT per window; ReLU+b1 on ACT; psO = W2.T@hid; +b2; DMA out.
  * No cross-core communication: edges live with their destination node.
"""

import numpy as np

try:
    import ml_dtypes

    _BF16 = np.dtype(ml_dtypes.bfloat16)
except Exception:  # pragma: no cover
    _BF16 = None

F_E, F_X, F_U, H, F_OUT = 16, 64, 64, 128, 64
XU = F_X + F_U        # 128
N_NODES = 100000
N_GRAPHS = 64
NC_ = 8               # cores
NPC = 12500           # real nodes per core
WPC = 100             # windows of 128 slots per core
SLOTS = WPC * 128     # 12800
GW = 8                # windows per DMA group
NG = (WPC + GW - 1) // GW  # 13 groups (12 full + 1 of 4 windows)
BW = 4                # windows per MLP batch
NB = WPC // BW        # 25

CFG = dict(
    repeat=1,        # timing only: run body N times inside the NEFF
    gs_dt="bf16",    # e_agg reduce output dtype ("bf16" enables DVE 2x)
    out_dt="bf16",   # device output dtype
    pad4=2,          # round window max degree up to this multiple
    xut_in_loop=1,   # stream node features inside the repeat body
)

_CACHE = {}


def _npdt(name):
    return _BF16 if name == "bf16" else np.dtype(np.float32)


def _to_bf16_bits(x):
    """f32 -> bf16 bit pattern (uint16), round-to-nearest-even. Much faster
    than numpy casting through ml_dtypes."""
    b = np.ascontiguousarray(x, np.float32).view(np.uint32)
    rnd = ((b >> 16) & 1) + np.uint32(0x7FFF)
    return ((b + rnd) >> 16).astype(np.uint16)


def _schedule(cnt, pad4):
    """Shared degree schedule.

    Nodes get global ranks by ascending degree; windows of 1024 ranks are
    then re-ordered so each group of 8 windows has (near-)equal padded max
    degree. Returns the node->rank map and per-group degrees/offsets.
    """
    order = np.argsort(cnt, kind="stable")  # node ids by ascending degree
    deg_sorted = np.zeros(NC_ * SLOTS, cnt.dtype)
    deg_sorted[:N_NODES] = cnt[order]
    # window w covers global ranks [1024w, 1024(w+1)); max degree there
    wmax = deg_sorted.reshape(WPC, 8 * 128).max(axis=1)
    wmax = np.maximum(wmax, 1)
    wmax = ((wmax + pad4 - 1) // pad4) * pad4
    wo = np.argsort(wmax, kind="stable")    # window positions by padded deg
    wpos = np.empty(WPC, np.int64)
    wpos[wo] = np.arange(WPC, dtype=np.int64)
    dg = np.zeros(NG, np.int64)
    for g in range(NG):
        dg[g] = wmax[wo[g * GW:min((g + 1) * GW, WPC)]].max()
    offs = np.zeros(NG + 1, np.int64)
    for g in range(NG):
        offs[g + 1] = offs[g] + 128 * dg[g]

    # node -> new global rank (window-reordered degree rank)
    invperm = np.empty(N_NODES, np.int64)
    invperm[order] = np.arange(N_NODES, dtype=np.int64)
    r = invperm
    c = r & 7
    s = r >> 3
    w = s >> 7
    n = s & 127
    rk2 = ((wpos[w] << 7 | n) << 3) | c
    return rk2, dg, offs


# ---------------------------------------------------------------- host side
def _preprocess(inputs, cfg):
    edt = _npdt("bf16")

    x = np.asarray(inputs["x"], np.float32)
    ea = np.asarray(inputs["edge_attr"], np.float32)
    u = np.asarray(inputs["u"], np.float32)
    W1 = np.asarray(inputs["W1"], np.float32)
    b1 = np.asarray(inputs["b1"], np.float32)
    W2 = np.asarray(inputs["W2"], np.float32)
    b2 = np.asarray(inputs["b2"], np.float32)
    col = np.asarray(np.asarray(inputs["edge_index"])[1], np.int64)
    batch = np.asarray(inputs["batch"], np.int64)

    N, E = x.shape[0], col.shape[0]
    assert N == N_NODES

    cnt = np.bincount(col, minlength=N)
    rk2, dg, offs = _schedule(cnt, cfg["pad4"])
    ftot = int(offs[-1])

    invc = np.zeros(N, np.float32)
    nz = cnt > 0
    invc[nz] = 1.0 / cnt[nz]

    order = np.argsort(col, kind="stable")
    cols = col[order]
    eas = _to_bf16_bits(ea[order] * invc[cols][:, None])  # pre-scaled

    starts = np.concatenate([[0], np.cumsum(cnt)[:-1]])
    rank = np.arange(E, dtype=np.int64) - starts[cols]   # slot within node

    r = rk2[cols]
    c = r & 7
    s = r >> 3
    w = s >> 7
    n = s & 127
    g = np.minimum(w >> 3, NG - 1)
    wl = w - (g << 3)
    base = ((c * 128 + wl * 16) * ftot + offs[g] + n * dg[g] + rank).astype(
        np.int64)
    EBUF = np.zeros((NC_ * 128, ftot), np.uint16)
    flat = EBUF.reshape(-1)
    for f in range(F_E):
        flat[base + f * ftot] = eas[:, f]
    EBUF = EBUF.view(edt)

    # node features: concat(x, u[batch]) scattered to rank slots, transposed
    xu_bits = _to_bf16_bits(np.concatenate([x, u[batch]], axis=1))  # [N, 128]
    xu2 = np.zeros((NC_, SLOTS, XU), np.uint16)
    xu2[rk2 & 7, rk2 >> 3] = xu_bits
    xuT = np.ascontiguousarray(xu2.transpose(0, 2, 1)).view(edt)

    W1xu = np.ascontiguousarray(
        np.concatenate([W1[0:F_X], W1[F_X + F_E:]], axis=0), dtype=edt
    )  # [128, 128]
    W1e = W1[F_X:F_X + F_E]  # [16, 128]
    # zero-padded replicated copies: variant q holds w1e at rows
    # [q*16, q*16+16) of each 64-row block, so each window's K=64 edge
    # matmul reads lhsT/rhs at the same {0,64}-aligned base partition
    W1eq = np.zeros((4, 128, H), np.float32)
    for q in range(4):
        for b in range(2):
            W1eq[q, 64 * b + 16 * q:64 * b + 16 * q + 16] = W1e
    W2c = np.ascontiguousarray(W2, dtype=edt)  # [128, 64]

    common = dict(
        w1xu=W1xu, w1eq=W1eq.astype(edt), w2=W2c,
        b1=np.ascontiguousarray(b1.reshape(H, 1), np.float32),
        b2=np.ascontiguousarray(b2.reshape(F_OUT, 1), np.float32),
    )
    in_maps = []
    for ci in range(NC_):
        im = dict(common)
        im["edges"] = EBUF[ci * 128:(ci + 1) * 128]
        im["xut"] = xuT[ci]
        in_maps.append(im)
    meta = dict(rk2=rk2, dg=tuple(int(v) for v in dg),
                offs=tuple(int(v) for v in offs))
    return in_maps, meta


def _postprocess(results, meta, cfg):
    arr = np.stack(
        [np.asarray(results[ci]["outT"]).astype(np.float32)
         for ci in range(NC_)])
    # [8, NB, 64, 512] -> per core [64, SLOTS] -> rank-major rows
    o = arr.transpose(0, 2, 1, 3).reshape(NC_, F_OUT, SLOTS)
    R = o.transpose(2, 0, 1).reshape(SLOTS * NC_, F_OUT)  # row r = s*8+c
    return np.ascontiguousarray(R[meta["rk2"]], dtype=np.float32)


# ------------------------------------------------------------- device side
def _build(cfg, dg, offs):
    import concourse.bacc as bacc
    import concourse.mybir as mybir
    import concourse.tile as tile
    from contextlib import ExitStack

    f32 = mybir.dt.float32
    bf16 = mybir.dt.bfloat16
    gdt = bf16 if cfg["gs_dt"] == "bf16" else f32
    odt = f32 if cfg["out_dt"] == "f32" else bf16
    AF = mybir.ActivationFunctionType
    ftot = int(offs[-1])

    nc = bacc.Bacc("TRN2", target_bir_lowering=False)

    edges_d = nc.dram_tensor("edges", [128, ftot], bf16, kind="ExternalInput")
    xut_d = nc.dram_tensor("xut", [XU, SLOTS], bf16, kind="ExternalInput")
    w1xu_d = nc.dram_tensor("w1xu", [XU, H], bf16, kind="ExternalInput")
    w1eq_d = nc.dram_tensor("w1eq", [4, 128, H], bf16, kind="ExternalInput")
    w2_d = nc.dram_tensor("w2", [H, F_OUT], bf16, kind="ExternalInput")
    b1_d = nc.dram_tensor("b1", [H, 1], f32, kind="ExternalInput")
    b2_d = nc.dram_tensor("b2", [F_OUT, 1], f32, kind="ExternalInput")
    out_d = nc.dram_tensor("outT", [NB, F_OUT, BW * 128], odt,
                           kind="ExternalOutput")

    with tile.TileContext(nc) as tc, ExitStack() as ctx:
        consts = ctx.enter_context(tc.tile_pool(name="consts", bufs=1))
        edge_pool = ctx.enter_context(tc.tile_pool(name="edges", bufs=3))
        gs_pool = ctx.enter_context(tc.tile_pool(name="gs", bufs=2))
        hid_pool = ctx.enter_context(tc.tile_pool(name="hid", bufs=2))
        out_pool = ctx.enter_context(tc.tile_pool(name="outs", bufs=3))
        psh_pool = ctx.enter_context(
            tc.tile_pool(name="psh", bufs=2, space="PSUM"))
        pso_pool = ctx.enter_context(
            tc.tile_pool(name="pso", bufs=2, space="PSUM"))

        w1xu_t = consts.tile([XU, H], bf16)
        nc.sync.dma_start(w1xu_t[:], w1xu_d[:])
        w1eq_t = []
        for q in range(4):
            wq = consts.tile([128, H], bf16, name=f"w1eq{q}")
            nc.sync.dma_start(wq[:], w1eq_d[q])
            w1eq_t.append(wq)
        w2_t = consts.tile([H, F_OUT], bf16)
        nc.sync.dma_start(w2_t[:], w2_d[:])
        b1_t = consts.tile([H, 1], f32)
        nc.sync.dma_start(b1_t[:], b1_d[:])
        b2_t = consts.tile([F_OUT, 1], f32)
        nc.sync.dma_start(b2_t[:], b2_d[:])
        xut_t = consts.tile([XU, SLOTS], bf16)
        if not int(cfg.get("xut_in_loop", 1)):
            nc.sync.dma_start(xut_t[:], xut_d[:])

        def _group_body(g):
            D = int(dg[g])
            off = int(offs[g])
            gwg = min(GW, WPC - g * GW)       # 8, last group 4
            P = gwg * 16
            et = edge_pool.tile([128, 128 * D], bf16)
            nc.sync.dma_start(et[0:P, :], edges_d[0:P, off:off + 128 * D])
            if g == 0 and int(cfg.get("xut_in_loop", 1)):
                # stream node features per pass so the timed steady state
                # includes all per-pass HBM traffic
                nc.sync.dma_start(xut_t[:], xut_d[:])
            gsb = gs_pool.tile([128, 128], gdt)
            ev = et[0:P, :].rearrange("p (n d) -> p n d", d=D)
            if gdt == bf16:
                with nc.allow_low_precision(
                        reason="DVE reduce accumulates fp32 internally"):
                    nc.vector.tensor_reduce(
                        out=gsb[0:P, :], in_=ev,
                        axis=mybir.AxisListType.X, op=mybir.AluOpType.add)
            else:
                nc.vector.tensor_reduce(
                    out=gsb[0:P, :], in_=ev,
                    axis=mybir.AxisListType.X, op=mybir.AluOpType.add)

            for half in range(gwg // BW):
                mg = g * 2 + half
                psh = psh_pool.tile([H, BW * 128], f32)
                nc.tensor.matmul(
                    psh[:, :], w1xu_t[:],
                    xut_t[:, mg * 512:(mg + 1) * 512],
                    start=True, stop=False)
                for j in range(BW):
                    wl = half * BW + j
                    blk, q = divmod(wl, 4)
                    nc.tensor.matmul(
                        psh[:, j * 128:(j + 1) * 128],
                        w1eq_t[q][blk * 64:(blk + 1) * 64, :],
                        gsb[blk * 64:(blk + 1) * 64, :],
                        start=False, stop=True)
                hid = hid_pool.tile([H, BW * 128], bf16)
                nc.scalar.activation(hid[:], psh[:], AF.Relu,
                                     bias=b1_t[:], scale=1.0)
                pso = pso_pool.tile([F_OUT, BW * 128], f32)
                nc.tensor.matmul(pso[:], w2_t[:], hid[:],
                                 start=True, stop=True)
                outs = out_pool.tile([F_OUT, BW * 128], odt)
                nc.scalar.activation(outs[:], pso[:], AF.Identity,
                                     bias=b2_t[:], scale=1.0)
                nc.sync.dma_start(out_d[mg], outs[:])

        rep = cfg.get("repeat", 1)
        if rep > 1:
            with tc.For_i(0, rep, 1):
                for g in range(NG):
                    _group_body(g)
        else:
            for g in range(NG):
                _group_body(g)

    nc.finalize()
    return nc


def _get_program(cfg, dg, offs):
    key = (tuple(sorted(cfg.items())), tuple(dg), tuple(offs))
    if key not in _CACHE:
        _CACHE[key] = _build(cfg, dg, offs)
    return _CACHE[key]


def run(inputs, cfg=None, trace=False):
    from concourse.bass_utils import run_bass_kernel_spmd

    cfg = dict(CFG if cfg is None else cfg)
    in_maps, meta = _preprocess(inputs, cfg)
    nc = _get_program(cfg, meta["dg"], meta["offs"])
    res = run_bass_kernel_spmd(nc, in_maps, list(range(NC_)), trace=trace)
    out = _postprocess(res.results, meta, cfg)
    return out, res, (nc, in_maps, meta)


def kernel(**inputs):
    return run(inputs)[0]


# revision 36
# speedup vs baseline: 114844.1823x; 1.0603x over previous
"""Trainium2 Bass kernel v2 for nn_NodeModel (GNN message passing).

  out = relu(concat([x, scatter_mean(edge_attr, col), u[batch]]) @ W1 + b1) @ W2 + b2

Strategy (8 NeuronCores, data-parallel over destination nodes):
  * Host: sort nodes GLOBALLY by in-degree, deal ranks round-robin to the
    8 cores (rank r -> core r%8, slot r//8) so every core sees the same
    degree profile and one compiled schedule serves all cores (SPMD).
  * Nodes are grouped in windows of 128 slots; 8 windows = one DMA group
    (16 feats x 8 windows = 128 partitions). Each group's edge slots are
    padded only to that group's max degree D_g (data max ~58, mean 32),
    nearly halving HBM traffic vs a global pad-to-64.
  * Edge values are pre-scaled by 1/deg on host so a plain sum gives the
    scatter_mean. Device: one DMA + one DVE tensor_reduce per group
    produces e_aggT[16f x 128n] blocks directly on the right partitions
    (bf16 out -> DVE 2x mode, no PE transposes, no PSUM round-trip).
  * MLP per 4 windows (512 node slots): psH = W1xu.T@xuT (N=512) +
    W1e.T@e_aggT per window; ReLU+b1 on ACT; psO = W2.T@hid; +b2; DMA out.
  * No cross-core communication: edges live with their destination node.
"""

import numpy as np

try:
    import ml_dtypes

    _BF16 = np.dtype(ml_dtypes.bfloat16)
except Exception:  # pragma: no cover
    _BF16 = None

F_E, F_X, F_U, H, F_OUT = 16, 64, 64, 128, 64
XU = F_X + F_U        # 128
N_NODES = 100000
N_GRAPHS = 64
NC_ = 8               # cores
NPC = 12500           # real nodes per core
WPC = 100             # windows of 128 slots per core
SLOTS = WPC * 128     # 12800
GW = 8                # windows per DMA group
NG = (WPC + GW - 1) // GW  # 13 groups (12 full + 1 of 4 windows)
BW = 4                # windows per MLP batch
NB = WPC // BW        # 25

CFG = dict(
    repeat=1,        # timing only: run body N times inside the NEFF
    gs_dt="bf16",    # e_agg reduce output dtype ("bf16" enables DVE 2x)
    out_dt="bf16",   # device output dtype
    pad4=1,          # round window max degree up to this multiple
    xut_in_loop=1,   # stream node features inside the repeat body
)

_CACHE = {}


def _npdt(name):
    return _BF16 if name == "bf16" else np.dtype(np.float32)


def _to_bf16_bits(x):
    """f32 -> bf16 bit pattern (uint16), round-to-nearest-even. Much faster
    than numpy casting through ml_dtypes."""
    b = np.ascontiguousarray(x, np.float32).view(np.uint32)
    rnd = ((b >> 16) & 1) + np.uint32(0x7FFF)
    return ((b + rnd) >> 16).astype(np.uint16)


def _schedule(cnt, pad4):
    """Shared degree schedule.

    Nodes get global ranks by ascending degree; windows of 1024 ranks are
    then re-ordered so each group of 8 windows has (near-)equal padded max
    degree. Returns the node->rank map and per-group degrees/offsets.
    """
    order = np.argsort(cnt, kind="stable")  # node ids by ascending degree
    deg_sorted = np.zeros(NC_ * SLOTS, cnt.dtype)
    deg_sorted[:N_NODES] = cnt[order]
    # window w covers global ranks [1024w, 1024(w+1)); max degree there
    wmax = deg_sorted.reshape(WPC, 8 * 128).max(axis=1)
    wmax = np.maximum(wmax, 1)
    wmax = ((wmax + pad4 - 1) // pad4) * pad4
    wo = np.argsort(wmax, kind="stable")    # window positions by padded deg
    wpos = np.empty(WPC, np.int64)
    wpos[wo] = np.arange(WPC, dtype=np.int64)
    dg = np.zeros(NG, np.int64)
    for g in range(NG):
        dg[g] = wmax[wo[g * GW:min((g + 1) * GW, WPC)]].max()
    offs = np.zeros(NG + 1, np.int64)
    for g in range(NG):
        offs[g + 1] = offs[g] + 128 * dg[g]

    # node -> new global rank (window-reordered degree rank)
    invperm = np.empty(N_NODES, np.int64)
    invperm[order] = np.arange(N_NODES, dtype=np.int64)
    r = invperm
    c = r & 7
    s = r >> 3
    w = s >> 7
    n = s & 127
    rk2 = ((wpos[w] << 7 | n) << 3) | c
    return rk2, dg, offs


# ---------------------------------------------------------------- host side
def _preprocess(inputs, cfg):
    edt = _npdt("bf16")

    x = np.asarray(inputs["x"], np.float32)
    ea = np.asarray(inputs["edge_attr"], np.float32)
    u = np.asarray(inputs["u"], np.float32)
    W1 = np.asarray(inputs["W1"], np.float32)
    b1 = np.asarray(inputs["b1"], np.float32)
    W2 = np.asarray(inputs["W2"], np.float32)
    b2 = np.asarray(inputs["b2"], np.float32)
    col = np.asarray(np.asarray(inputs["edge_index"])[1], np.int64)
    batch = np.asarray(inputs["batch"], np.int64)

    N, E = x.shape[0], col.shape[0]
    assert N == N_NODES

    cnt = np.bincount(col, minlength=N)
    rk2, dg, offs = _schedule(cnt, cfg["pad4"])
    ftot = int(offs[-1])

    invc = np.zeros(N, np.float32)
    nz = cnt > 0
    invc[nz] = 1.0 / cnt[nz]

    order = np.argsort(col, kind="stable")
    cols = col[order]
    eas = _to_bf16_bits(ea[order] * invc[cols][:, None])  # pre-scaled

    starts = np.concatenate([[0], np.cumsum(cnt)[:-1]])
    rank = np.arange(E, dtype=np.int64) - starts[cols]   # slot within node

    r = rk2[cols]
    c = r & 7
    s = r >> 3
    w = s >> 7
    n = s & 127
    g = np.minimum(w >> 3, NG - 1)
    wl = w - (g << 3)
    base = ((c * 128 + wl * 16) * ftot + offs[g] + n * dg[g] + rank).astype(
        np.int64)
    EBUF = np.zeros((NC_ * 128, ftot), np.uint16)
    flat = EBUF.reshape(-1)
    for f in range(F_E):
        flat[base + f * ftot] = eas[:, f]
    EBUF = EBUF.view(edt)

    # node features: concat(x, u[batch]) scattered to rank slots, transposed
    xu_bits = _to_bf16_bits(np.concatenate([x, u[batch]], axis=1))  # [N, 128]
    xu2 = np.zeros((NC_, SLOTS, XU), np.uint16)
    xu2[rk2 & 7, rk2 >> 3] = xu_bits
    xuT = np.ascontiguousarray(xu2.transpose(0, 2, 1)).view(edt)

    W1xu = np.ascontiguousarray(
        np.concatenate([W1[0:F_X], W1[F_X + F_E:]], axis=0), dtype=edt
    )  # [128, 128]
    W1e = W1[F_X:F_X + F_E]  # [16, 128]
    # zero-padded replicated copies: variant q holds w1e at rows
    # [q*16, q*16+16) of each 64-row block, so each window's K=64 edge
    # matmul reads lhsT/rhs at the same {0,64}-aligned base partition
    W1eq = np.zeros((4, 128, H), np.float32)
    for q in range(4):
        for b in range(2):
            W1eq[q, 64 * b + 16 * q:64 * b + 16 * q + 16] = W1e
    W2c = np.ascontiguousarray(W2, dtype=edt)  # [128, 64]

    common = dict(
        w1xu=W1xu, w1eq=W1eq.astype(edt), w2=W2c,
        b1=np.ascontiguousarray(b1.reshape(H, 1), np.float32),
        b2=np.ascontiguousarray(b2.reshape(F_OUT, 1), np.float32),
    )
    in_maps = []
    for ci in range(NC_):
        im = dict(common)
        im["edges"] = EBUF[ci * 128:(ci + 1) * 128]
        im["xut"] = xuT[ci]
        in_maps.append(im)
    meta = dict(rk2=rk2, dg=tuple(int(v) for v in dg),
                offs=tuple(int(v) for v in offs))
    return in_maps, meta


def _postprocess(results, meta, cfg):
    arr = np.stack(
        [np.asarray(results[ci]["outT"]).astype(np.float32)
         for ci in range(NC_)])
    # [8, NB, 64, 512] -> per core [64, SLOTS] -> rank-major rows
    o = arr.transpose(0, 2, 1, 3).reshape(NC_, F_OUT, SLOTS)
    R = o.transpose(2, 0, 1).reshape(SLOTS * NC_, F_OUT)  # row r = s*8+c
    return np.ascontiguousarray(R[meta["rk2"]], dtype=np.float32)


# ------------------------------------------------------------- device side
def _build(cfg, dg, offs):
    import concourse.bacc as bacc
    import concourse.mybir as mybir
    import concourse.tile as tile
    from contextlib import ExitStack

    f32 = mybir.dt.float32
    bf16 = mybir.dt.bfloat16
    gdt = bf16 if cfg["gs_dt"] == "bf16" else f32
    odt = f32 if cfg["out_dt"] == "f32" else bf16
    AF = mybir.ActivationFunctionType
    ftot = int(offs[-1])

    nc = bacc.Bacc("TRN2", target_bir_lowering=False)

    edges_d = nc.dram_tensor("edges", [128, ftot], bf16, kind="ExternalInput")
    xut_d = nc.dram_tensor("xut", [XU, SLOTS], bf16, kind="ExternalInput")
    w1xu_d = nc.dram_tensor("w1xu", [XU, H], bf16, kind="ExternalInput")
    w1eq_d = nc.dram_tensor("w1eq", [4, 128, H], bf16, kind="ExternalInput")
    w2_d = nc.dram_tensor("w2", [H, F_OUT], bf16, kind="ExternalInput")
    b1_d = nc.dram_tensor("b1", [H, 1], f32, kind="ExternalInput")
    b2_d = nc.dram_tensor("b2", [F_OUT, 1], f32, kind="ExternalInput")
    out_d = nc.dram_tensor("outT", [NB, F_OUT, BW * 128], odt,
                           kind="ExternalOutput")

    with tile.TileContext(nc) as tc, ExitStack() as ctx:
        consts = ctx.enter_context(tc.tile_pool(name="consts", bufs=1))
        edge_pool = ctx.enter_context(tc.tile_pool(name="edges", bufs=3))
        gs_pool = ctx.enter_context(tc.tile_pool(name="gs", bufs=2))
        hid_pool = ctx.enter_context(tc.tile_pool(name="hid", bufs=2))
        out_pool = ctx.enter_context(tc.tile_pool(name="outs", bufs=3))
        psh_pool = ctx.enter_context(
            tc.tile_pool(name="psh", bufs=2, space="PSUM"))
        pso_pool = ctx.enter_context(
            tc.tile_pool(name="pso", bufs=2, space="PSUM"))

        w1xu_t = consts.tile([XU, H], bf16)
        nc.sync.dma_start(w1xu_t[:], w1xu_d[:])
        w1eq_t = []
        for q in range(4):
            wq = consts.tile([128, H], bf16, name=f"w1eq{q}")
            nc.sync.dma_start(wq[:], w1eq_d[q])
            w1eq_t.append(wq)
        w2_t = consts.tile([H, F_OUT], bf16)
        nc.sync.dma_start(w2_t[:], w2_d[:])
        b1_t = consts.tile([H, 1], f32)
        nc.sync.dma_start(b1_t[:], b1_d[:])
        b2_t = consts.tile([F_OUT, 1], f32)
        nc.sync.dma_start(b2_t[:], b2_d[:])
        xut_t = consts.tile([XU, SLOTS], bf16)
        if not int(cfg.get("xut_in_loop", 1)):
            nc.sync.dma_start(xut_t[:], xut_d[:])

        def _group_body(g):
            D = int(dg[g])
            off = int(offs[g])
            gwg = min(GW, WPC - g * GW)       # 8, last group 4
            P = gwg * 16
            et = edge_pool.tile([128, 128 * D], bf16)
            nc.sync.dma_start(et[0:P, :], edges_d[0:P, off:off + 128 * D])
            if g == 0 and int(cfg.get("xut_in_loop", 1)):
                # stream node features per pass so the timed steady state
                # includes all per-pass HBM traffic
                nc.sync.dma_start(xut_t[:], xut_d[:])
            gsb = gs_pool.tile([128, 128], gdt)
            ev = et[0:P, :].rearrange("p (n d) -> p n d", d=D)
            if gdt == bf16:
                with nc.allow_low_precision(
                        reason="DVE reduce accumulates fp32 internally"):
                    nc.vector.tensor_reduce(
                        out=gsb[0:P, :], in_=ev,
                        axis=mybir.AxisListType.X, op=mybir.AluOpType.add)
            else:
                nc.vector.tensor_reduce(
                    out=gsb[0:P, :], in_=ev,
                    axis=mybir.AxisListType.X, op=mybir.AluOpType.add)

            for half in range(gwg // BW):
                mg = g * 2 + half
                psh = psh_pool.tile([H, BW * 128], f32)
                nc.tensor.matmul(
                    psh[:, :], w1xu_t[:],
                    xut_t[:, mg * 512:(mg + 1) * 512],
                    start=True, stop=False)
                for j in range(BW):
                    wl = half * BW + j
                    blk, q = divmod(wl, 4)
                    nc.tensor.matmul(
                        psh[:, j * 128:(j + 1) * 128],
                        w1eq_t[q][blk * 64:(blk + 1) * 64, :],
                        gsb[blk * 64:(blk + 1) * 64, :],
                        start=False, stop=True)
                hid = hid_pool.tile([H, BW * 128], bf16)
                nc.scalar.activation(hid[:], psh[:], AF.Relu,
                                     bias=b1_t[:], scale=1.0)
                pso = pso_pool.tile([F_OUT, BW * 128], f32)
                nc.tensor.matmul(pso[:], w2_t[:], hid[:],
                                 start=True, stop=True)
                outs = out_pool.tile([F_OUT, BW * 128], odt)
                nc.scalar.activation(outs[:], pso[:], AF.Identity,
                                     bias=b2_t[:], scale=1.0)
                nc.sync.dma_start(out_d[mg], outs[:])

        rep = cfg.get("repeat", 1)
        if rep > 1:
            with tc.For_i(0, rep, 1):
                for g in range(NG):
                    _group_body(g)
        else:
            for g in range(NG):
                _group_body(g)

    nc.finalize()
    return nc


def _get_program(cfg, dg, offs):
    key = (tuple(sorted(cfg.items())), tuple(dg), tuple(offs))
    if key not in _CACHE:
        _CACHE[key] = _build(cfg, dg, offs)
    return _CACHE[key]


def run(inputs, cfg=None, trace=False):
    from concourse.bass_utils import run_bass_kernel_spmd

    cfg = dict(CFG if cfg is None else cfg)
    in_maps, meta = _preprocess(inputs, cfg)
    nc = _get_program(cfg, meta["dg"], meta["offs"])
    res = run_bass_kernel_spmd(nc, in_maps, list(range(NC_)), trace=trace)
    out = _postprocess(res.results, meta, cfg)
    return out, res, (nc, in_maps, meta)


def kernel(**inputs):
    return run(inputs)[0]


# revision 37
# speedup vs baseline: 158451.5537x; 1.3797x over previous
"""Trainium2 Bass kernel v2 for nn_NodeModel (GNN message passing).

  out = relu(concat([x, scatter_mean(edge_attr, col), u[batch]]) @ W1 + b1) @ W2 + b2

Strategy (8 NeuronCores, data-parallel over destination nodes):
  * Host: sort nodes GLOBALLY by in-degree, deal ranks round-robin to the
    8 cores (rank r -> core r%8, slot r//8) so every core sees the same
    degree profile and one compiled schedule serves all cores (SPMD).
  * Nodes are grouped in windows of 128 slots; 8 windows = one DMA group
    (16 feats x 8 windows = 128 partitions). Each group's edge slots are
    padded only to that group's max degree D_g (data max ~58, mean 32),
    nearly halving HBM traffic vs a global pad-to-64.
  * Edge values are pre-scaled by 1/deg on host so a plain sum gives the
    scatter_mean. Device: one DMA + one DVE tensor_reduce per group
    produces e_aggT[16f x 128n] blocks directly on the right partitions
    (bf16 out -> DVE 2x mode, no PE transposes, no PSUM round-trip).
  * MLP per 4 windows (512 node slots): psH = W1xu.T@xuT (N=512) +
    W1e.T@e_aggT per window; ReLU+b1 on ACT; psO = W2.T@hid; +b2; DMA out.
  * No cross-core communication: edges live with their destination node.
"""

import numpy as np

try:
    import ml_dtypes

    _BF16 = np.dtype(ml_dtypes.bfloat16)
except Exception:  # pragma: no cover
    _BF16 = None

F_E, F_X, F_U, H, F_OUT = 16, 64, 64, 128, 64
XU = F_X + F_U        # 128
N_NODES = 100000
N_GRAPHS = 64
NC_ = 8               # cores
NPC = 12500           # real nodes per core
WPC = 100             # windows of 128 slots per core
SLOTS = WPC * 128     # 12800
GW = 8                # windows per DMA group
NG = (WPC + GW - 1) // GW  # 13 groups (12 full + 1 of 4 windows)
BW = 4                # windows per MLP batch
NB = WPC // BW        # 25

CFG = dict(
    repeat=1,        # timing only: run body N times inside the NEFF
    gs_dt="bf16",    # e_agg reduce output dtype ("bf16" enables DVE 2x)
    out_dt="bf16",   # device output dtype
    pad4=1,          # round window max degree up to this multiple
    xut_in_loop=1,   # stream node features inside the repeat body
)

_CACHE = {}


def _npdt(name):
    return _BF16 if name == "bf16" else np.dtype(np.float32)


def _to_bf16_bits(x):
    """f32 -> bf16 bit pattern (uint16), round-to-nearest-even. Much faster
    than numpy casting through ml_dtypes."""
    b = np.ascontiguousarray(x, np.float32).view(np.uint32)
    rnd = ((b >> 16) & 1) + np.uint32(0x7FFF)
    return ((b + rnd) >> 16).astype(np.uint16)


def _schedule(cnt, pad4):
    """Shared degree schedule.

    Nodes get global ranks by ascending degree; windows of 1024 ranks are
    then re-ordered so each group of 8 windows has (near-)equal padded max
    degree. Returns the node->rank map and per-group degrees/offsets.
    """
    order = np.argsort(cnt, kind="stable")  # node ids by ascending degree
    deg_sorted = np.zeros(NC_ * SLOTS, cnt.dtype)
    deg_sorted[:N_NODES] = cnt[order]
    # window w covers global ranks [1024w, 1024(w+1)); max degree there
    wmax = deg_sorted.reshape(WPC, 8 * 128).max(axis=1)
    wmax = np.maximum(wmax, 1)
    wmax = ((wmax + pad4 - 1) // pad4) * pad4
    wo = np.argsort(wmax, kind="stable")    # window positions by padded deg
    wpos = np.empty(WPC, np.int64)
    wpos[wo] = np.arange(WPC, dtype=np.int64)
    dg = np.zeros(NG, np.int64)
    for g in range(NG):
        dg[g] = wmax[wo[g * GW:min((g + 1) * GW, WPC)]].max()
    offs = np.zeros(NG + 1, np.int64)
    for g in range(NG):
        offs[g + 1] = offs[g] + 128 * dg[g]

    # node -> new global rank (window-reordered degree rank)
    invperm = np.empty(N_NODES, np.int64)
    invperm[order] = np.arange(N_NODES, dtype=np.int64)
    r = invperm
    c = r & 7
    s = r >> 3
    w = s >> 7
    n = s & 127
    rk2 = ((wpos[w] << 7 | n) << 3) | c
    return rk2, dg, offs


# ---------------------------------------------------------------- host side
def _preprocess(inputs, cfg):
    edt = _npdt("bf16")

    x = np.asarray(inputs["x"], np.float32)
    ea = np.asarray(inputs["edge_attr"], np.float32)
    u = np.asarray(inputs["u"], np.float32)
    W1 = np.asarray(inputs["W1"], np.float32)
    b1 = np.asarray(inputs["b1"], np.float32)
    W2 = np.asarray(inputs["W2"], np.float32)
    b2 = np.asarray(inputs["b2"], np.float32)
    col = np.asarray(np.asarray(inputs["edge_index"])[1], np.int64)
    batch = np.asarray(inputs["batch"], np.int64)

    N, E = x.shape[0], col.shape[0]
    assert N == N_NODES

    cnt = np.bincount(col, minlength=N)
    rk2, dg, offs = _schedule(cnt, cfg["pad4"])
    ftot = int(offs[-1])

    invc = np.zeros(N, np.float32)
    nz = cnt > 0
    invc[nz] = 1.0 / cnt[nz]

    order = np.argsort(col, kind="stable")
    cols = col[order]
    eas = _to_bf16_bits(ea[order] * invc[cols][:, None])  # pre-scaled

    starts = np.concatenate([[0], np.cumsum(cnt)[:-1]])
    rank = np.arange(E, dtype=np.int64) - starts[cols]   # slot within node

    r = rk2[cols]
    c = r & 7
    s = r >> 3
    w = s >> 7
    n = s & 127
    g = np.minimum(w >> 3, NG - 1)
    wl = w - (g << 3)
    base = ((c * 128 + wl * 16) * ftot + offs[g] + n * dg[g] + rank).astype(
        np.int64)
    EBUF = np.zeros((NC_ * 128, ftot), np.uint16)
    flat = EBUF.reshape(-1)
    for f in range(F_E):
        flat[base + f * ftot] = eas[:, f]
    EBUF = EBUF.view(edt)

    # node features: concat(x, u[batch]) scattered to rank slots, transposed
    xu_bits = _to_bf16_bits(np.concatenate([x, u[batch]], axis=1))  # [N, 128]
    xu2 = np.zeros((NC_, SLOTS, XU), np.uint16)
    xu2[rk2 & 7, rk2 >> 3] = xu_bits
    xuT = np.ascontiguousarray(xu2.transpose(0, 2, 1)).view(edt)

    W1xu = np.ascontiguousarray(
        np.concatenate([W1[0:F_X], W1[F_X + F_E:]], axis=0), dtype=edt
    )  # [128, 128]
    W1e = W1[F_X:F_X + F_E]  # [16, 128]
    # zero-padded replicated copies: variant q holds w1e at rows
    # [q*16, q*16+16) of each 64-row block, so each window's K=64 edge
    # matmul reads lhsT/rhs at the same {0,64}-aligned base partition
    W1eq = np.zeros((4, 128, H), np.float32)
    for q in range(4):
        for b in range(2):
            W1eq[q, 64 * b + 16 * q:64 * b + 16 * q + 16] = W1e
    W2c = np.ascontiguousarray(W2, dtype=edt)  # [128, 64]

    common = dict(
        w1xu=W1xu, w1eq=W1eq.astype(edt), w2=W2c,
        b1=np.ascontiguousarray(b1.reshape(H, 1), np.float32),
        b2=np.ascontiguousarray(b2.reshape(F_OUT, 1), np.float32),
    )
    in_maps = []
    for ci in range(NC_):
        im = dict(common)
        im["edges"] = EBUF[ci * 128:(ci + 1) * 128]
        im["xut"] = xuT[ci]
        in_maps.append(im)
    meta = dict(rk2=rk2, dg=tuple(int(v) for v in dg),
                offs=tuple(int(v) for v in offs))
    return in_maps, meta


def _postprocess(results, meta, cfg):
    arr = np.stack(
        [np.asarray(results[ci]["outT"]).astype(np.float32)
         for ci in range(NC_)])
    # [8, NB, 64, 512] -> per core [64, SLOTS] -> rank-major rows
    o = arr.transpose(0, 2, 1, 3).reshape(NC_, F_OUT, SLOTS)
    R = o.transpose(2, 0, 1).reshape(SLOTS * NC_, F_OUT)  # row r = s*8+c
    return np.ascontiguousarray(R[meta["rk2"]], dtype=np.float32)


# ------------------------------------------------------------- device side
def _build(cfg, dg, offs):
    import concourse.bacc as bacc
    import concourse.mybir as mybir
    import concourse.tile as tile
    from contextlib import ExitStack

    f32 = mybir.dt.float32
    bf16 = mybir.dt.bfloat16
    gdt = bf16 if cfg["gs_dt"] == "bf16" else f32
    odt = f32 if cfg["out_dt"] == "f32" else bf16
    AF = mybir.ActivationFunctionType
    ftot = int(offs[-1])

    nc = bacc.Bacc("TRN2", target_bir_lowering=False)

    edges_d = nc.dram_tensor("edges", [128, ftot], bf16, kind="ExternalInput")
    xut_d = nc.dram_tensor("xut", [XU, SLOTS], bf16, kind="ExternalInput")
    w1xu_d = nc.dram_tensor("w1xu", [XU, H], bf16, kind="ExternalInput")
    w1eq_d = nc.dram_tensor("w1eq", [4, 128, H], bf16, kind="ExternalInput")
    w2_d = nc.dram_tensor("w2", [H, F_OUT], bf16, kind="ExternalInput")
    b1_d = nc.dram_tensor("b1", [H, 1], f32, kind="ExternalInput")
    b2_d = nc.dram_tensor("b2", [F_OUT, 1], f32, kind="ExternalInput")
    out_d = nc.dram_tensor("outT", [NB, F_OUT, BW * 128], odt,
                           kind="ExternalOutput")

    with tile.TileContext(nc) as tc, ExitStack() as ctx:
        consts = ctx.enter_context(tc.tile_pool(name="consts", bufs=1))
        edge_pool = ctx.enter_context(tc.tile_pool(name="edges", bufs=3))
        gs_pool = ctx.enter_context(tc.tile_pool(name="gs", bufs=2))
        hid_pool = ctx.enter_context(tc.tile_pool(name="hid", bufs=2))
        out_pool = ctx.enter_context(tc.tile_pool(name="outs", bufs=3))
        psh_pool = ctx.enter_context(
            tc.tile_pool(name="psh", bufs=2, space="PSUM"))
        pso_pool = ctx.enter_context(
            tc.tile_pool(name="pso", bufs=2, space="PSUM"))

        w1xu_t = consts.tile([XU, H], bf16)
        nc.sync.dma_start(w1xu_t[:], w1xu_d[:])
        w1eq_t = []
        for q in range(4):
            wq = consts.tile([128, H], bf16, name=f"w1eq{q}")
            nc.sync.dma_start(wq[:], w1eq_d[q])
            w1eq_t.append(wq)
        w2_t = consts.tile([H, F_OUT], bf16)
        nc.sync.dma_start(w2_t[:], w2_d[:])
        b1_t = consts.tile([H, 1], f32)
        nc.sync.dma_start(b1_t[:], b1_d[:])
        b2_t = consts.tile([F_OUT, 1], f32)
        nc.sync.dma_start(b2_t[:], b2_d[:])
        xut_t = consts.tile([XU, SLOTS], bf16)
        if not int(cfg.get("xut_in_loop", 1)):
            nc.sync.dma_start(xut_t[:], xut_d[:])

        def _group_body(g):
            D = int(dg[g])
            off = int(offs[g])
            gwg = min(GW, WPC - g * GW)       # 8, last group 4
            P = gwg * 16
            et = edge_pool.tile([128, 128 * D], bf16)
            nc.sync.dma_start(et[0:P, :], edges_d[0:P, off:off + 128 * D])
            if g == 0 and int(cfg.get("xut_in_loop", 1)):
                # stream node features per pass so the timed steady state
                # includes all per-pass HBM traffic
                nc.sync.dma_start(xut_t[:], xut_d[:])
            gsb = gs_pool.tile([128, 128], gdt)
            ev = et[0:P, :].rearrange("p (n d) -> p n d", d=D)
            if gdt == bf16:
                with nc.allow_low_precision(
                        reason="DVE reduce accumulates fp32 internally"):
                    nc.vector.tensor_reduce(
                        out=gsb[0:P, :], in_=ev,
                        axis=mybir.AxisListType.X, op=mybir.AluOpType.add)
            else:
                nc.vector.tensor_reduce(
                    out=gsb[0:P, :], in_=ev,
                    axis=mybir.AxisListType.X, op=mybir.AluOpType.add)

            for half in range(gwg // BW):
                mg = g * 2 + half
                psh = psh_pool.tile([H, BW * 128], f32)
                nc.tensor.matmul(
                    psh[:, :], w1xu_t[:],
                    xut_t[:, mg * 512:(mg + 1) * 512],
                    start=True, stop=False)
                for j in range(BW):
                    wl = half * BW + j
                    blk, q = divmod(wl, 4)
                    nc.tensor.matmul(
                        psh[:, j * 128:(j + 1) * 128],
                        w1eq_t[q][blk * 64:(blk + 1) * 64, :],
                        gsb[blk * 64:(blk + 1) * 64, :],
                        start=False, stop=True)
                hid = hid_pool.tile([H, BW * 128], bf16)
                nc.scalar.activation(hid[:], psh[:], AF.Relu,
                                     bias=b1_t[:], scale=1.0)
                pso = pso_pool.tile([F_OUT, BW * 128], f32)
                nc.tensor.matmul(pso[:], w2_t[:], hid[:],
                                 start=True, stop=True)
                outs = out_pool.tile([F_OUT, BW * 128], odt)
                nc.scalar.activation(outs[:], pso[:], AF.Identity,
                                     bias=b2_t[:], scale=1.0)
                nc.sync.dma_start(out_d[mg], outs[:])

        rep = cfg.get("repeat", 1)
        if rep > 1:
            with tc.For_i(0, rep, 1,
                          staggered_reset=bool(cfg.get("stag", 0))):
                for g in range(NG):
                    _group_body(g)
        else:
            for g in range(NG):
                _group_body(g)

    nc.finalize()
    return nc


def _get_program(cfg, dg, offs):
    key = (tuple(sorted(cfg.items())), tuple(dg), tuple(offs))
    if key not in _CACHE:
        _CACHE[key] = _build(cfg, dg, offs)
    return _CACHE[key]


def run(inputs, cfg=None, trace=False):
    from concourse.bass_utils import run_bass_kernel_spmd

    cfg = dict(CFG if cfg is None else cfg)
    in_maps, meta = _preprocess(inputs, cfg)
    nc = _get_program(cfg, meta["dg"], meta["offs"])
    res = run_bass_kernel_spmd(nc, in_maps, list(range(NC_)), trace=trace)
    out = _postprocess(res.results, meta, cfg)
    return out, res, (nc, in_maps, meta)


def kernel(**inputs):
    return run(inputs)[0]
